# revision 29
# baseline (speedup 1.0000x reference)
"""Trainium2 Bass kernel for nn_Bottleneck_75325136437765 (sparse 3x3 local attention bottleneck).

Sharding: data-parallel over batch B=16 across 8 cores (2 batches/core), params replicated.

v4: software-pipelined two-batch schedule. Channels on partitions, spatial on free dim,
all matmuls bf16 with fp32 PSUM.

Per batch:
  conv1 / q,k convs / v conv / conv3: plain matmuls (host-pretransposed weights,
      bn scales folded). The v conv runs off the critical path (as PE filler).
  logits, packed by di (rows 32*dj+head, 96 rows per di-tile): qpos matmuls (P2)
      + 3-shift-batched q*k products on DVE + 0/1-selection matmuls via tile_position.
  softmax: exp on ACT (3 ops of 96 rows); den via 0/1 matmuls; reciprocal on DVE;
      1/den applied at the end in channel space (bf16 broadcast).
  v-apply: e head->channel broadcast via SBUF-SBUF DMA issued right after each
      exp (spread across gpsimd/sync/scalar rings), 3-shift-batched products on DVE,
      sum over shifts via identity-matmul PSUM accumulation (two accumulators:
      pacc for mc=0, the logits bank for mc=1).
  output: z = conv3 + b3 (no relu) -> bf16 -> DRAM; HOST computes relu(z + x).

Pipeline (emission order == per-engine execution order):
  A(b0) conv1 | Bqk(b0) | C(b0) logits + [vconv(b0), A(b1)] filler
  | D(b0) v-apply + [Bqk(b1), vconv(b1)] filler | C(b1) + conv3(b0,0-3)
  | D(b1) + conv3(b0,4-7) | conv3(b1).
"""

import itertools

import numpy as np

import concourse.bass as bass
import concourse.bacc as bacc
import concourse.tile as tile
from concourse import mybir
from concourse.bass_utils import run_bass_kernel_spmd

# ---- problem constants (hardcoded per contract) ----
B, CIN, H, W = 16, 1024, 32, 32
WIDTH, OUT, HEADS, KS = 256, 1024, 32, 3
D = WIDTH // HEADS            # 8 channels per head
HW = H * W                    # 1024
NC_ = 8                       # cores
BL = B // NC_                 # 2 batches per core
P = 128
KC1 = CIN // P                # 8 contraction chunks for conv1
PT = WIDTH // P               # 2 partition tiles for width-256 tensors
OC = OUT // P                 # 8 output ptiles for conv3
NKK = KS * KS                 # 9 shifts
RQ = KS * HEADS               # 96 packed logit rows per di-tile
F32 = mybir.dt.float32
BF16 = mybir.dt.bfloat16
NHALF = 2                     # PSUM-bank limit: matmul N<=512 fp32 out

# packed fp32 consts layout (free-dim offsets in cstf)
_CF = {"b1": 0, "bq": 2, "bk": 4, "bv": 6, "batt": 8, "b3": 10}
CF_N = 18
# packed bf16 consts layout
_CB = {"sel": 0, "sab": 64, "ident": 128}
CB_N = 256


def _ns(n):
    return slice(n * 512, (n + 1) * 512)


def build_program():
    nc = bacc.Bacc(None, target_bir_lowering=False, debug=False)

    x16_d = nc.dram_tensor("x16", [BL, KC1, P, HW], BF16, kind="ExternalInput").ap()
    w1T_d = nc.dram_tensor("w1T", [P, KC1, WIDTH], BF16, kind="ExternalInput").ap()
    wqT_d = nc.dram_tensor("wqT", [P, PT, WIDTH], BF16, kind="ExternalInput").ap()
    wkT_d = nc.dram_tensor("wkT", [P, PT, WIDTH], BF16, kind="ExternalInput").ap()
    wvT_d = nc.dram_tensor("wvT", [P, PT, WIDTH], BF16, kind="ExternalInput").ap()
    w3T_d = nc.dram_tensor("w3T", [P, PT, OUT], BF16, kind="ExternalInput").ap()
    p2_d = nc.dram_tensor("p2", [P, PT, KS, RQ], BF16, kind="ExternalInput").ap()
    cstf_d = nc.dram_tensor("cstf", [P, CF_N], F32, kind="ExternalInput").ap()
    cstb_d = nc.dram_tensor("cstb", [P, CB_N], BF16, kind="ExternalInput").ap()
    out_d = nc.dram_tensor("out", [BL, OC, P, HW], BF16, kind="ExternalOutput").ap()

    with tile.TileContext(nc) as tc:
        with (
            tc.tile_pool(name="consts", bufs=1) as consts,
            tc.tile_pool(name="xb", bufs=2) as xbp,
            tc.tile_pool(name="act", bufs=2) as actp,
            tc.tile_pool(name="attn", bufs=2) as attnp,
            tc.tile_pool(name="epk", bufs=3) as epkp,
            tc.tile_pool(name="tmp", bufs=2) as tmpp,
            tc.tile_pool(name="tmp2", bufs=3) as tmp2p,
            tc.tile_pool(name="ebm", bufs=6) as ebmp,
            tc.tile_pool(name="outz", bufs=8) as outzp,
            tc.tile_pool(name="pmm", bufs=2, space="PSUM") as pmm,
            tc.tile_pool(name="pL", bufs=1, space="PSUM") as pLp,
            tc.tile_pool(name="pacc", bufs=1, space="PSUM") as paccp,
        ):
            # ---- constants (SWDGE/gpsimd queue; sync queue serves x first) ----
            w1T = consts.tile([P, KC1, WIDTH], BF16, tag="w1T")
            wqT = consts.tile([P, PT, WIDTH], BF16, tag="wqT")
            wkT = consts.tile([P, PT, WIDTH], BF16, tag="wkT")
            wvT = consts.tile([P, PT, WIDTH], BF16, tag="wvT")
            w3T = consts.tile([P, PT, OUT], BF16, tag="w3T")
            p2 = consts.tile([P, PT, KS, RQ], BF16, tag="p2")
            cstf = consts.tile([P, CF_N], F32, tag="cstf")
            cstb = consts.tile([P, CB_N], BF16, tag="cstb")
            nc.scalar.dma_start(out=w1T, in_=w1T_d)
            nc.gpsimd.dma_start(out=wqT, in_=wqT_d)
            nc.gpsimd.dma_start(out=wkT, in_=wkT_d)
            nc.gpsimd.dma_start(out=wvT, in_=wvT_d)
            nc.gpsimd.dma_start(out=w3T, in_=w3T_d)
            nc.gpsimd.dma_start(out=p2, in_=p2_d)
            nc.gpsimd.dma_start(out=cstf, in_=cstf_d)
            nc.gpsimd.dma_start(out=cstb, in_=cstb_d)

            def cf(name, npt):  # fp32 const slice as [P, npt, 1]
                o = _CF[name]
                return cstf[:, o:o + npt].rearrange("p (k m) -> p k m", m=1)

            b1, bq, bk, bv, batt = (cf(n, PT) for n in ("b1", "bq", "bk", "bv", "batt"))
            b3 = cf("b3", OC)
            sel = cstb[:, _CB["sel"]:_CB["sel"] + PT * HEADS].rearrange(
                "p (k m) -> p k m", k=PT)
            sab = cstb[:, _CB["sab"]:_CB["sab"] + HEADS]
            ident = cstb[:, _CB["ident"]:_CB["ident"] + P]

            def head_bcast_dma(dst, src16, eng):
                # dst[g*8+d, :] = src16[g, :] — 2-level partition AP broadcast
                bc = bass.AP(tensor=src16.tensor, offset=src16.offset,
                             ap=[list(src16.ap[0]), [0, D]]
                                + [list(a) for a in src16.ap[1:]])
                eng.dma_start(out=dst, in_=bc)

            # persistent zero-padded k/v tiles, one per batch (borders stay 0)
            kpad = [consts.tile([P, PT, H + 2, W + 2], BF16, tag=f"kpad{b}",
                                name=f"kpad{b}") for b in range(BL)]
            vpad = [consts.tile([P, PT, H + 2, W + 2], BF16, tag=f"vpad{b}",
                                name=f"vpad{b}") for b in range(BL)]
            for b in range(BL):
                nc.gpsimd.memset(kpad[b], 0.0)
                nc.gpsimd.memset(vpad[b], 0.0)

            # ---- x loads: both batches early, 2 chunks each ----
            xb = []
            for b in range(BL):
                t = xbp.tile([P, KC1, HW], BF16, tag="xb")
                for lo, hi in ((0, 1), (1, 2), (2, 4), (4, 6), (6, 8)):
                    nc.sync.dma_start(
                        out=t[:, lo:hi, :],
                        in_=x16_d[b, lo:hi].rearrange("k p m -> p k m"))
                xb.append(t)

            # ---- per-batch state ----
            h1 = [None] * BL
            q = [None] * BL
            h2 = [None] * BL
            recip_bc = [None] * BL
            # ebm[b][mc][di]: [P, 3(dj), HW] broadcast-e tiles
            ebm = [[[None] * KS for _ in range(PT)] for _ in range(BL)]

            # ======== phase emitters (generators yield at PE-interleave points) ====

            def conv1_gen(b):
                h1[b] = actp.tile([P, PT, HW], BF16, tag="h1", name=f"h1_{b}")
                for mc in range(PT):
                    ps = pmm.tile([P, HW], F32, tag="mm")
                    for kc in range(KC1):
                        for n in range(NHALF):
                            nc.tensor.matmul(
                                ps[:, _ns(n)],
                                w1T[:, kc, mc * P:(mc + 1) * P],
                                xb[b][:, kc, _ns(n)],
                                start=(kc == 0), stop=(kc == KC1 - 1),
                            )
                        yield
                    nc.scalar.activation(
                        out=h1[b][:, mc, :], in_=ps,
                        func=mybir.ActivationFunctionType.Relu,
                        bias=b1[:, mc], scale=1.0)
                    yield

            def _conv256(b, wT, bias, relu, dest, qdest):
                for mc in range(PT):
                    ps = pmm.tile([P, HW], F32, tag="mm")
                    for kc in range(PT):
                        for n in range(NHALF):
                            nc.tensor.matmul(
                                ps[:, _ns(n)],
                                wT[:, kc, mc * P:(mc + 1) * P],
                                h1[b][:, kc, _ns(n)],
                                start=(kc == 0), stop=(kc == PT - 1),
                            )
                        yield
                    if dest is None:
                        o, i = qdest[:, mc, :], ps[:]
                    else:
                        o = dest[:, mc, 1:H + 1, 1:W + 1]
                        i = ps.rearrange("p (a b) -> p a b", a=H)
                    nc.scalar.activation(
                        out=o, in_=i,
                        func=(mybir.ActivationFunctionType.Relu if relu
                              else mybir.ActivationFunctionType.Identity),
                        bias=bias[:, mc], scale=1.0)
                    yield

            def qk_gen(b):
                q[b] = actp.tile([P, PT, HW], BF16, tag="q", name=f"q_{b}")
                yield from _conv256(b, wqT, bq, True, None, q[b])
                yield from _conv256(b, wkT, bk, True, kpad[b], None)

            def vconv_gen(b):
                yield from _conv256(b, wvT, bv, False, vpad[b], None)

            def conv3_gen(b, ocs, defer=None, mix_zr=False):
                # z = conv3(h2) + b3 (no relu, no residual — host finishes)
                for oc in ocs:
                    ps = pmm.tile([P, HW], F32, tag="mm")
                    for n in range(NHALF):
                        for kc in range(PT):
                            nc.tensor.matmul(
                                ps[:, _ns(n)],
                                w3T[:, kc, oc * P:(oc + 1) * P],
                                h2[b][:, kc, _ns(n)],
                                start=(kc == 0), stop=(kc == PT - 1),
                                skip_group_check=True,
                            )
                        yield
                    zr = outzp.tile([P, HW], BF16, tag="outzr")
                    if mix_zr and oc % 2 == 1:
                        # keep the tail ACT queue short: odd ocs on DVE
                        nc.vector.tensor_scalar_add(zr, ps, b3[:, oc])
                    else:
                        nc.scalar.activation(
                            out=zr, in_=ps,
                            func=mybir.ActivationFunctionType.Identity,
                            bias=b3[:, oc], scale=1.0)
                    if defer is None:
                        nc.sync.dma_start(out=out_d[b, oc], in_=zr)
                    else:
                        defer.append((b, oc, zr))
                    yield

            def flush_gen(defer):
                while defer:
                    b_, oc_, zr_ = defer.pop(0)
                    nc.sync.dma_start(out=out_d[b_, oc_], in_=zr_)
                    yield

            def _fill(filler, k=1):
                if filler is None:
                    return
                for _ in range(k):
                    if next(filler, "END") == "END":
                        return

            def drain(filler):
                if filler is not None:
                    for _ in filler:
                        pass

            eb_rings = [nc.gpsimd, nc.sync, nc.scalar, nc.sync, nc.gpsimd,
                        nc.sync]

            def logits(b, filler):
                """qpos matmuls + 3-shift products (DVE) + sel matmuls (PE)
                + exp (ACT) + den + recip + broadcasts (DMA, prefetched)."""
                denp = paccp.tile([HEADS, HW], F32, tag="acc")
                nring = 0
                for mc in range(PT):
                    for di in range(KS):
                        ebm[b][mc][di] = ebmp.tile(
                            [P, KS, HW], BF16, tag="ebm",
                            name=f"ebm_{b}_{mc}_{di}")
                for di in range(KS):
                    Lpk = pLp.tile([P, HW], F32, tag="Lpk")
                    # qpos term: all 96 rows at once per pt chunk
                    for n in range(NHALF):
                        for pt in range(PT):
                            nc.tensor.matmul(
                                Lpk[:RQ, _ns(n)],
                                p2[:, pt, di, :],
                                q[b][:, pt, _ns(n)],
                                start=(pt == 0), stop=False,
                                skip_group_check=True,
                            )
                    _fill(filler, 1)
                    # 3-shift-batched qk products + col-tiled group reduce
                    for pt in range(PT):
                        tmp3 = tmpp.tile([P, KS, HW], BF16, tag="tmp")
                        kp = kpad[b][:, pt]  # [P, 34, 34]
                        in0 = bass.AP(
                            tensor=kp.tensor, offset=kp.offset + di * (W + 2),
                            ap=[list(kp.ap[0]), [1, KS], [W + 2, H], [1, W]])
                        qv = q[b][:, pt, :]
                        in1 = bass.AP(
                            tensor=qv.tensor, offset=qv.offset,
                            ap=[list(qv.ap[0]), [0, KS], [W, H], [1, W]])
                        nc.vector.tensor_tensor(
                            out=tmp3.rearrange("p k (a b) -> p k a b", a=H),
                            in0=in0, in1=in1, op=mybir.AluOpType.mult)
                        for dj in range(KS):
                            for n in range(NHALF):
                                nc.tensor.matmul(
                                    Lpk[32 * dj:32 * (dj + 1), _ns(n)],
                                    sel[:, pt, :],
                                    tmp3[:, dj, _ns(n)],
                                    start=False, stop=(pt == PT - 1),
                                    tile_position=(0, 32 * dj),
                                    skip_group_check=True,
                                )
                            _fill(filler, 1)
                    epk = epkp.tile([P, HW], BF16, tag="epk")
                    nc.scalar.activation(
                        out=epk[:RQ, :], in_=Lpk[:RQ, :],
                        func=mybir.ActivationFunctionType.Exp)
                    # prefetch e broadcasts for this di (round-robin rings so
                    # descriptor generation isn't serialized)
                    for dj in range(KS):
                        for mc in range(PT):
                            r0 = 32 * dj + 16 * mc
                            head_bcast_dma(
                                ebm[b][mc][di][:, dj, :],
                                epk[r0:r0 + 16, :],
                                eb_rings[nring % len(eb_rings)])
                            nring += 1
                    # denominator accumulation
                    for n in range(NHALF):
                        nc.tensor.matmul(
                            denp[:, _ns(n)], sab[:RQ, :], epk[:RQ, _ns(n)],
                            start=(di == 0), stop=(di == KS - 1),
                            skip_group_check=True,
                        )
                    _fill(filler, 2)
                denf = attnp.tile([HEADS, HW], F32, tag="denf")
                nc.vector.reciprocal_approx_fast(out=denf, in_=denp)
                den = attnp.tile([HEADS, HW], BF16, tag="den")
                nc.scalar.activation(
                    out=den, in_=denf,
                    func=mybir.ActivationFunctionType.Identity, scale=1.0)
                recip_bc[b] = attnp.tile([P, PT, HW], BF16, tag="recip_bc",
                                         name=f"recip_bc_{b}")
                for mc in range(PT):
                    head_bcast_dma(recip_bc[b][:, mc, :],
                                   den[16 * mc:16 * (mc + 1), :], nc.sync)

            def vside(b, filler):
                h2[b] = actp.tile([P, PT, HW], BF16, tag="h2", name=f"h2_{b}")
                for mc in range(PT):
                    # two accumulators: pacc (mc=0) and the logits bank (mc=1)
                    pool = paccp if mc == 0 else pLp
                    tg = "acc" if mc == 0 else "Lpk"
                    acc = pool.tile([P, HW], F32, tag=tg, name=f"acc_{b}_{mc}")
                    for di in range(KS):
                        # 3-shift-batched product: t2[p,dj,hw] = e*v_shift
                        t2 = tmp2p.tile([P, KS, HW], BF16, tag="tmp2")
                        vsh = vpad[b][:, mc]  # [P, 34, 34]
                        in1 = bass.AP(
                            tensor=vsh.tensor, offset=vsh.offset + di * (W + 2),
                            ap=[list(vsh.ap[0]), [1, KS], [W + 2, H], [1, W]])
                        nc.vector.tensor_tensor(
                            out=t2.rearrange("p k (a b) -> p k a b", a=H),
                            in0=ebm[b][mc][di].rearrange(
                                "p k (a b) -> p k a b", a=H),
                            in1=in1,
                            op=mybir.AluOpType.mult,
                        )
                        for dj in range(KS):
                            kk = KS * di + dj
                            for n in range(NHALF):
                                nc.tensor.matmul(
                                    acc[:, _ns(n)], ident, t2[:, dj, _ns(n)],
                                    start=(kk == 0), stop=(kk == NKK - 1),
                                    skip_group_check=True,
                                )
                        _fill(filler, 2)
                    # h2 = relu(acc) * recip_bc (bnatt bias is zero in this
                    # problem so relu commutes with the positive 1/den factor;
                    # _host_prep asserts it)
                    h2r = tmp2p.tile([P, HW], BF16, tag="t3")
                    nc.scalar.activation(
                        out=h2r, in_=acc,
                        func=mybir.ActivationFunctionType.Relu, scale=1.0)
                    nc.vector.tensor_tensor(
                        out=h2[b][:, mc, :], in0=h2r,
                        in1=recip_bc[b][:, mc, :],
                        op=mybir.AluOpType.mult,
                    )
                    _fill(filler, 2)

            # ======== pipelined schedule ========
            drain(conv1_gen(0))                       # A(b0)
            drain(qk_gen(0))                          # Bqk(b0)
            f = itertools.chain(vconv_gen(0), conv1_gen(1))
            logits(0, f)                              # C(b0) + fill
            drain(f)
            f = itertools.chain(qk_gen(1), vconv_gen(1))
            vside(0, f)                               # D(b0) + fill
            drain(f)
            deferred = []
            f = conv3_gen(0, range(0, 4), defer=deferred)
            logits(1, f)                              # C(b1) + E(b0) 0-3
            drain(f)
            f = itertools.chain(conv3_gen(0, range(4, OC), defer=deferred),
                                flush_gen(deferred))
            vside(1, f)                               # D(b1) + E(b0) 4-7
            drain(f)
            drain(conv3_gen(1, range(OC), mix_zr=True))  # E(b1)

    nc.compile()
    return nc


_PROG = None


def _host_prep(inputs):
    import ml_dtypes
    bf = ml_dtypes.bfloat16
    f = lambda a: np.asarray(a, dtype=np.float32)
    x = f(inputs["x"])
    # fold bn scales into weights (bn(conv(x,W),s,b) = conv(x, s*W) + b)
    w1 = f(inputs["w_conv1"]) * f(inputs["bn1_s"])[:, None]
    wq = f(inputs["wq"]) * f(inputs["bnq_s"])[:, None]
    wk = f(inputs["wk"]) * f(inputs["bnk_s"])[:, None]
    # fold bnatt scale through the (linear) attention-value path into v
    sv = f(inputs["bnatt_s"]) * f(inputs["bnv_s"])
    wv = f(inputs["wv"]) * sv[:, None]
    bv = f(inputs["bnatt_s"]) * f(inputs["bnv_b"])
    w3 = f(inputs["w_conv3"]) * f(inputs["bn3_s"])[:, None]

    posf = (f(inputs["pos_h"]) + f(inputs["pos_w"])).reshape(WIDTH, NKK)

    def pt_major(w, ko, no):  # [no, ko] -> [P, ko/P, no] partition-major
        return np.ascontiguousarray(
            w.T.reshape(ko // P, P, no).transpose(1, 0, 2))

    sel = np.zeros((PT, P, HEADS), np.float32)
    for pt in range(PT):
        for c in range(P):
            sel[pt, c, pt * (P // D) + c // D] = 1.0
    sab = np.zeros((P, HEADS), np.float32)
    for r in range(P):
        sab[r, r % HEADS] = 1.0
    # p2[p, pt, di, 32*dj+g] = pos[pt*128+p, 3di+dj] if head(pt*128+p)==g
    p2 = np.zeros((PT, P, KS, RQ), np.float32)
    for pt in range(PT):
        for c in range(P):
            g = pt * (P // D) + c // D
            for kk in range(NKK):
                di, dj = kk // KS, kk % KS
                p2[pt, c, di, 32 * dj + g % HEADS] = posf[pt * P + c, kk]
    p2 = np.ascontiguousarray(p2.transpose(1, 0, 2, 3))

    # h2 relu/recip commute relies on zero bnatt bias (see vside tail)
    assert np.allclose(f(inputs["bnatt_b"]), 0.0), "bnatt_b must be zero"
    cstf = np.zeros((P, CF_N), np.float32)
    for name, arr, npt in (("b1", f(inputs["bn1_b"]), PT),
                           ("bq", f(inputs["bnq_b"]), PT),
                           ("bk", f(inputs["bnk_b"]), PT),
                           ("bv", bv, PT),
                           ("batt", f(inputs["bnatt_b"]), PT),
                           ("b3", f(inputs["bn3_b"]), OC)):
        cstf[:, _CF[name]:_CF[name] + npt] = arr.reshape(npt, P).T

    cstb = np.zeros((P, CB_N), np.float32)
    cstb[:, _CB["sel"]:_CB["sel"] + PT * HEADS] = (
        sel.transpose(1, 0, 2).reshape(P, PT * HEADS))
    cstb[:, _CB["sab"]:_CB["sab"] + HEADS] = sab
    cstb[:, _CB["ident"]:_CB["ident"] + P] = np.eye(P)

    com = {
        "w1T": pt_major(w1, CIN, WIDTH).astype(bf),
        "wqT": pt_major(wq, WIDTH, WIDTH).astype(bf),
        "wkT": pt_major(wk, WIDTH, WIDTH).astype(bf),
        "wvT": pt_major(wv, WIDTH, WIDTH).astype(bf),
        "w3T": pt_major(w3, WIDTH, OUT).astype(bf),
        "p2": p2.astype(bf),
        "cstf": cstf,
        "cstb": cstb.astype(bf),
    }
    xr = x.reshape(B, KC1, P, HW)
    in_maps = []
    for c in range(NC_):
        xs = np.ascontiguousarray(xr[c * BL:(c + 1) * BL])
        in_maps.append(dict(com, x16=xs.astype(bf)))
    return in_maps


def _finish(raw_outs, x):
    """Host-side tail: y = relu(z + x) with z the device output (conv3+b3)."""
    z = np.concatenate(
        [o.astype(np.float32).reshape(BL, OUT, H, W) for o in raw_outs], axis=0)
    return np.maximum(z + np.asarray(x, dtype=np.float32), 0.0)


def kernel(**inputs):
    global _PROG
    if _PROG is None:
        _PROG = build_program()
    in_maps = _host_prep(inputs)
    res = run_bass_kernel_spmd(_PROG, in_maps, core_ids=list(range(NC_)))
    return _finish([res.results[c]["out"] for c in range(NC_)], inputs["x"])


# revision 30
# speedup vs baseline: 1.0684x; 1.0684x over previous
"""Trainium2 Bass kernel for nn_Bottleneck_75325136437765 (sparse 3x3 local attention bottleneck).

Sharding: data-parallel over batch B=16 across 8 cores (2 batches/core), params replicated.

v4: software-pipelined two-batch schedule. Channels on partitions, spatial on free dim,
all matmuls bf16 with fp32 PSUM.

Per batch:
  conv1 / q,k convs / v conv / conv3: plain matmuls (host-pretransposed weights,
      bn scales folded). The v conv runs off the critical path (as PE filler).
  logits, packed by di (rows 32*dj+head, 96 rows per di-tile): qpos matmuls (P2)
      + 3-shift-batched q*k products on DVE + 0/1-selection matmuls via tile_position.
  softmax: exp on ACT (3 ops of 96 rows); den via 0/1 matmuls; reciprocal on DVE;
      1/den applied at the end in channel space (bf16 broadcast).
  v-apply: e head->channel broadcast via SBUF-SBUF DMA issued right after each
      exp (spread across gpsimd/sync/scalar rings), 3-shift-batched products on DVE,
      sum over shifts via identity-matmul PSUM accumulation (two accumulators:
      pacc for mc=0, the logits bank for mc=1).
  output: z = conv3 + b3 (no relu) -> bf16 -> DRAM; HOST computes relu(z + x).

Pipeline (emission order == per-engine execution order):
  A(b0) conv1 | Bqk(b0) | C(b0) logits + [vconv(b0), A(b1)] filler
  | D(b0) v-apply + [Bqk(b1), vconv(b1)] filler | C(b1) + conv3(b0,0-3)
  | D(b1) + conv3(b0,4-7) | conv3(b1).
"""

import itertools

import numpy as np

import concourse.bass as bass
import concourse.bacc as bacc
import concourse.tile as tile
from concourse import mybir
from concourse.bass_utils import run_bass_kernel_spmd

# ---- problem constants (hardcoded per contract) ----
B, CIN, H, W = 16, 1024, 32, 32
WIDTH, OUT, HEADS, KS = 256, 1024, 32, 3
D = WIDTH // HEADS            # 8 channels per head
HW = H * W                    # 1024
NC_ = 8                       # cores
BL = B // NC_                 # 2 batches per core
P = 128
KC1 = CIN // P                # 8 contraction chunks for conv1
PT = WIDTH // P               # 2 partition tiles for width-256 tensors
OC = OUT // P                 # 8 output ptiles for conv3
NKK = KS * KS                 # 9 shifts
RQ = KS * HEADS               # 96 packed logit rows per di-tile
F32 = mybir.dt.float32
BF16 = mybir.dt.bfloat16
NHALF = 2                     # PSUM-bank limit: matmul N<=512 fp32 out

# packed fp32 consts layout (free-dim offsets in cstf)
_CF = {"b1": 0, "bq": 2, "bk": 4, "bv": 6, "batt": 8, "b3": 10}
CF_N = 18
# packed bf16 consts layout
_CB = {"sel": 0, "sab": 64, "ident": 128}
CB_N = 256


def _ns(n):
    return slice(n * 512, (n + 1) * 512)


def build_program():
    nc = bacc.Bacc(None, target_bir_lowering=False, debug=False)

    x16_d = nc.dram_tensor("x16", [BL, KC1, P, HW], BF16, kind="ExternalInput").ap()
    w1T_d = nc.dram_tensor("w1T", [P, KC1, WIDTH], BF16, kind="ExternalInput").ap()
    wqT_d = nc.dram_tensor("wqT", [P, PT, WIDTH], BF16, kind="ExternalInput").ap()
    wkT_d = nc.dram_tensor("wkT", [P, PT, WIDTH], BF16, kind="ExternalInput").ap()
    wvT_d = nc.dram_tensor("wvT", [P, PT, WIDTH], BF16, kind="ExternalInput").ap()
    w3T_d = nc.dram_tensor("w3T", [P, PT, OUT], BF16, kind="ExternalInput").ap()
    p2_d = nc.dram_tensor("p2", [P, PT, KS, RQ], BF16, kind="ExternalInput").ap()
    cstf_d = nc.dram_tensor("cstf", [P, CF_N], F32, kind="ExternalInput").ap()
    cstb_d = nc.dram_tensor("cstb", [P, CB_N], BF16, kind="ExternalInput").ap()
    out_d = nc.dram_tensor("out", [BL, OC, P, HW], BF16, kind="ExternalOutput").ap()

    with tile.TileContext(nc) as tc:
        with (
            tc.tile_pool(name="consts", bufs=1) as consts,
            tc.tile_pool(name="xb", bufs=2) as xbp,
            tc.tile_pool(name="act", bufs=2) as actp,
            tc.tile_pool(name="attn", bufs=2) as attnp,
            tc.tile_pool(name="epk", bufs=3) as epkp,
            tc.tile_pool(name="tmp", bufs=2) as tmpp,
            tc.tile_pool(name="tmp2", bufs=3) as tmp2p,
            tc.tile_pool(name="ebm", bufs=6) as ebmp,
            tc.tile_pool(name="outz", bufs=8) as outzp,
            tc.tile_pool(name="pmm", bufs=2, space="PSUM") as pmm,
            tc.tile_pool(name="pL", bufs=1, space="PSUM") as pLp,
            tc.tile_pool(name="pacc", bufs=1, space="PSUM") as paccp,
        ):
            # ---- constants (SWDGE/gpsimd queue; sync queue serves x first) ----
            w1T = consts.tile([P, KC1, WIDTH], BF16, tag="w1T")
            wqT = consts.tile([P, PT, WIDTH], BF16, tag="wqT")
            wkT = consts.tile([P, PT, WIDTH], BF16, tag="wkT")
            wvT = consts.tile([P, PT, WIDTH], BF16, tag="wvT")
            w3T = consts.tile([P, PT, OUT], BF16, tag="w3T")
            p2 = consts.tile([P, PT, KS, RQ], BF16, tag="p2")
            cstf = consts.tile([P, CF_N], F32, tag="cstf")
            cstb = consts.tile([P, CB_N], BF16, tag="cstb")
            nc.scalar.dma_start(out=w1T, in_=w1T_d)
            nc.gpsimd.dma_start(out=wqT, in_=wqT_d)
            nc.gpsimd.dma_start(out=wkT, in_=wkT_d)
            nc.gpsimd.dma_start(out=wvT, in_=wvT_d)
            nc.gpsimd.dma_start(out=w3T, in_=w3T_d)
            nc.gpsimd.dma_start(out=p2, in_=p2_d)
            nc.gpsimd.dma_start(out=cstf, in_=cstf_d)
            nc.gpsimd.dma_start(out=cstb, in_=cstb_d)

            def cf(name, npt):  # fp32 const slice as [P, npt, 1]
                o = _CF[name]
                return cstf[:, o:o + npt].rearrange("p (k m) -> p k m", m=1)

            b1, bq, bk, bv, batt = (cf(n, PT) for n in ("b1", "bq", "bk", "bv", "batt"))
            b3 = cf("b3", OC)
            sel = cstb[:, _CB["sel"]:_CB["sel"] + PT * HEADS].rearrange(
                "p (k m) -> p k m", k=PT)
            sab = cstb[:, _CB["sab"]:_CB["sab"] + HEADS]
            ident = cstb[:, _CB["ident"]:_CB["ident"] + P]

            def head_bcast_dma(dst, src16, eng):
                # dst[g*8+d, :] = src16[g, :] — 2-level partition AP broadcast
                bc = bass.AP(tensor=src16.tensor, offset=src16.offset,
                             ap=[list(src16.ap[0]), [0, D]]
                                + [list(a) for a in src16.ap[1:]])
                eng.dma_start(out=dst, in_=bc)

            # persistent zero-padded k/v tiles, one per batch (borders stay 0)
            kpad = [consts.tile([P, PT, H + 2, W + 2], BF16, tag=f"kpad{b}",
                                name=f"kpad{b}") for b in range(BL)]
            vpad = [consts.tile([P, PT, H + 2, W + 2], BF16, tag=f"vpad{b}",
                                name=f"vpad{b}") for b in range(BL)]
            for b in range(BL):
                nc.gpsimd.memset(kpad[b], 0.0)
                nc.gpsimd.memset(vpad[b], 0.0)

            # ---- x loads: both batches early, 2 chunks each ----
            xb = []
            for b in range(BL):
                t = xbp.tile([P, KC1, HW], BF16, tag="xb")
                for lo, hi in ((0, 1), (1, 2), (2, 4), (4, 6), (6, 8)):
                    nc.sync.dma_start(
                        out=t[:, lo:hi, :],
                        in_=x16_d[b, lo:hi].rearrange("k p m -> p k m"))
                xb.append(t)

            # ---- per-batch state ----
            h1 = [None] * BL
            q = [None] * BL
            h2 = [None] * BL
            recip_bc = [None] * BL
            # ebm[b][mc][di]: [P, 3(dj), HW] broadcast-e tiles
            ebm = [[[None] * KS for _ in range(PT)] for _ in range(BL)]

            # ======== phase emitters (generators yield at PE-interleave points) ====

            def conv1_gen(b):
                h1[b] = actp.tile([P, PT, HW], BF16, tag="h1", name=f"h1_{b}")
                for mc in range(PT):
                    ps = pmm.tile([P, HW], F32, tag="mm")
                    for kc in range(KC1):
                        for n in range(NHALF):
                            nc.tensor.matmul(
                                ps[:, _ns(n)],
                                w1T[:, kc, mc * P:(mc + 1) * P],
                                xb[b][:, kc, _ns(n)],
                                start=(kc == 0), stop=(kc == KC1 - 1),
                            )
                        yield
                    nc.scalar.activation(
                        out=h1[b][:, mc, :], in_=ps,
                        func=mybir.ActivationFunctionType.Relu,
                        bias=b1[:, mc], scale=1.0)
                    yield

            def _conv256(b, wT, bias, relu, dest, qdest):
                for mc in range(PT):
                    ps = pmm.tile([P, HW], F32, tag="mm")
                    for kc in range(PT):
                        for n in range(NHALF):
                            nc.tensor.matmul(
                                ps[:, _ns(n)],
                                wT[:, kc, mc * P:(mc + 1) * P],
                                h1[b][:, kc, _ns(n)],
                                start=(kc == 0), stop=(kc == PT - 1),
                            )
                        yield
                    if dest is None:
                        o, i = qdest[:, mc, :], ps[:]
                    else:
                        o = dest[:, mc, 1:H + 1, 1:W + 1]
                        i = ps.rearrange("p (a b) -> p a b", a=H)
                    nc.scalar.activation(
                        out=o, in_=i,
                        func=(mybir.ActivationFunctionType.Relu if relu
                              else mybir.ActivationFunctionType.Identity),
                        bias=bias[:, mc], scale=1.0)
                    yield

            def qk_gen(b):
                q[b] = actp.tile([P, PT, HW], BF16, tag="q", name=f"q_{b}")
                yield from _conv256(b, wqT, bq, True, None, q[b])
                yield from _conv256(b, wkT, bk, True, kpad[b], None)

            def vconv_gen(b):
                yield from _conv256(b, wvT, bv, False, vpad[b], None)

            def conv3_gen(b, ocs, defer=None, mix_zr=False):
                # z = conv3(h2) + b3 (no relu, no residual — host finishes)
                for oc in ocs:
                    ps = pmm.tile([P, HW], F32, tag="mm")
                    for n in range(NHALF):
                        for kc in range(PT):
                            nc.tensor.matmul(
                                ps[:, _ns(n)],
                                w3T[:, kc, oc * P:(oc + 1) * P],
                                h2[b][:, kc, _ns(n)],
                                start=(kc == 0), stop=(kc == PT - 1),
                                skip_group_check=True,
                            )
                        yield
                    zr = outzp.tile([P, HW], BF16, tag="outzr")
                    if mix_zr and oc % 2 == 1:
                        # keep the tail ACT queue short: odd ocs on DVE
                        nc.vector.tensor_scalar_add(zr, ps, b3[:, oc])
                    else:
                        nc.scalar.activation(
                            out=zr, in_=ps,
                            func=mybir.ActivationFunctionType.Identity,
                            bias=b3[:, oc], scale=1.0)
                    if defer is None:
                        nc.sync.dma_start(out=out_d[b, oc], in_=zr)
                    else:
                        defer.append((b, oc, zr))
                    yield

            def flush_gen(defer):
                while defer:
                    b_, oc_, zr_ = defer.pop(0)
                    nc.sync.dma_start(out=out_d[b_, oc_], in_=zr_)
                    yield

            def _fill(filler, k=1):
                if filler is None:
                    return
                for _ in range(k):
                    if next(filler, "END") == "END":
                        return

            def drain(filler):
                if filler is not None:
                    for _ in filler:
                        pass

            eb_rings = [nc.gpsimd, nc.sync, nc.gpsimd, nc.scalar, nc.gpsimd,
                        nc.sync]

            def logits(b, filler):
                """qpos matmuls + 3-shift products (DVE) + sel matmuls (PE)
                + exp (ACT) + den + recip + broadcasts (DMA, prefetched)."""
                denp = paccp.tile([HEADS, HW], F32, tag="acc")
                nring = 0
                for mc in range(PT):
                    for di in range(KS):
                        ebm[b][mc][di] = ebmp.tile(
                            [P, KS, HW], BF16, tag="ebm",
                            name=f"ebm_{b}_{mc}_{di}")
                for di in range(KS):
                    Lpk = pLp.tile([P, HW], F32, tag="Lpk")
                    # qpos term: all 96 rows at once per pt chunk
                    for n in range(NHALF):
                        for pt in range(PT):
                            nc.tensor.matmul(
                                Lpk[:RQ, _ns(n)],
                                p2[:, pt, di, :],
                                q[b][:, pt, _ns(n)],
                                start=(pt == 0), stop=False,
                                skip_group_check=True,
                            )
                    _fill(filler, 1)
                    # 3-shift-batched qk products + col-tiled group reduce
                    for pt in range(PT):
                        tmp3 = tmpp.tile([P, KS, HW], BF16, tag="tmp")
                        kp = kpad[b][:, pt]  # [P, 34, 34]
                        in0 = bass.AP(
                            tensor=kp.tensor, offset=kp.offset + di * (W + 2),
                            ap=[list(kp.ap[0]), [1, KS], [W + 2, H], [1, W]])
                        qv = q[b][:, pt, :]
                        in1 = bass.AP(
                            tensor=qv.tensor, offset=qv.offset,
                            ap=[list(qv.ap[0]), [0, KS], [W, H], [1, W]])
                        nc.vector.tensor_tensor(
                            out=tmp3.rearrange("p k (a b) -> p k a b", a=H),
                            in0=in0, in1=in1, op=mybir.AluOpType.mult)
                        for dj in range(KS):
                            for n in range(NHALF):
                                nc.tensor.matmul(
                                    Lpk[32 * dj:32 * (dj + 1), _ns(n)],
                                    sel[:, pt, :],
                                    tmp3[:, dj, _ns(n)],
                                    start=False, stop=(pt == PT - 1),
                                    tile_position=(0, 32 * dj),
                                    skip_group_check=True,
                                )
                            _fill(filler, 1)
                    epk = epkp.tile([P, HW], BF16, tag="epk")
                    nc.scalar.activation(
                        out=epk[:RQ, :], in_=Lpk[:RQ, :],
                        func=mybir.ActivationFunctionType.Exp)
                    # prefetch e broadcasts for this di (round-robin rings so
                    # descriptor generation isn't serialized)
                    for dj in range(KS):
                        for mc in range(PT):
                            r0 = 32 * dj + 16 * mc
                            head_bcast_dma(
                                ebm[b][mc][di][:, dj, :],
                                epk[r0:r0 + 16, :],
                                eb_rings[nring % len(eb_rings)])
                            nring += 1
                    # denominator accumulation
                    for n in range(NHALF):
                        nc.tensor.matmul(
                            denp[:, _ns(n)], sab[:RQ, :], epk[:RQ, _ns(n)],
                            start=(di == 0), stop=(di == KS - 1),
                            skip_group_check=True,
                        )
                    _fill(filler, 2)
                denf = attnp.tile([HEADS, HW], F32, tag="denf")
                nc.vector.reciprocal_approx_fast(out=denf, in_=denp)
                den = attnp.tile([HEADS, HW], BF16, tag="den")
                nc.scalar.activation(
                    out=den, in_=denf,
                    func=mybir.ActivationFunctionType.Identity, scale=1.0)
                recip_bc[b] = attnp.tile([P, PT, HW], BF16, tag="recip_bc",
                                         name=f"recip_bc_{b}")
                for mc in range(PT):
                    head_bcast_dma(recip_bc[b][:, mc, :],
                                   den[16 * mc:16 * (mc + 1), :], nc.sync)

            def vside(b, filler):
                h2[b] = actp.tile([P, PT, HW], BF16, tag="h2", name=f"h2_{b}")
                for mc in range(PT):
                    # two accumulators: pacc (mc=0) and the logits bank (mc=1)
                    pool = paccp if mc == 0 else pLp
                    tg = "acc" if mc == 0 else "Lpk"
                    acc = pool.tile([P, HW], F32, tag=tg, name=f"acc_{b}_{mc}")
                    for di in range(KS):
                        # 3-shift-batched product: t2[p,dj,hw] = e*v_shift
                        t2 = tmp2p.tile([P, KS, HW], BF16, tag="tmp2")
                        vsh = vpad[b][:, mc]  # [P, 34, 34]
                        in1 = bass.AP(
                            tensor=vsh.tensor, offset=vsh.offset + di * (W + 2),
                            ap=[list(vsh.ap[0]), [1, KS], [W + 2, H], [1, W]])
                        nc.vector.tensor_tensor(
                            out=t2.rearrange("p k (a b) -> p k a b", a=H),
                            in0=ebm[b][mc][di].rearrange(
                                "p k (a b) -> p k a b", a=H),
                            in1=in1,
                            op=mybir.AluOpType.mult,
                        )
                        for dj in range(KS):
                            kk = KS * di + dj
                            for n in range(NHALF):
                                nc.tensor.matmul(
                                    acc[:, _ns(n)], ident, t2[:, dj, _ns(n)],
                                    start=(kk == 0), stop=(kk == NKK - 1),
                                    skip_group_check=True,
                                )
                        _fill(filler, 2)
                    # h2 = relu(acc) * recip_bc (bnatt bias is zero in this
                    # problem so relu commutes with the positive 1/den factor;
                    # _host_prep asserts it)
                    h2r = tmp2p.tile([P, HW], BF16, tag="t3")
                    nc.scalar.activation(
                        out=h2r, in_=acc,
                        func=mybir.ActivationFunctionType.Relu, scale=1.0)
                    nc.vector.tensor_tensor(
                        out=h2[b][:, mc, :], in0=h2r,
                        in1=recip_bc[b][:, mc, :],
                        op=mybir.AluOpType.mult,
                    )
                    _fill(filler, 2)

            # ======== pipelined schedule ========
            drain(conv1_gen(0))                       # A(b0)
            drain(qk_gen(0))                          # Bqk(b0)
            f = itertools.chain(vconv_gen(0), conv1_gen(1))
            logits(0, f)                              # C(b0) + fill
            drain(f)
            f = itertools.chain(qk_gen(1), vconv_gen(1))
            vside(0, f)                               # D(b0) + fill
            drain(f)
            deferred = []
            f = conv3_gen(0, range(0, 4), defer=deferred)
            logits(1, f)                              # C(b1) + E(b0) 0-3
            drain(f)
            f = itertools.chain(conv3_gen(0, range(4, OC), defer=deferred),
                                flush_gen(deferred))
            vside(1, f)                               # D(b1) + E(b0) 4-7
            drain(f)
            drain(conv3_gen(1, range(OC), mix_zr=True))  # E(b1)

    nc.compile()
    return nc


_PROG = None


def _host_prep(inputs):
    import ml_dtypes
    bf = ml_dtypes.bfloat16
    f = lambda a: np.asarray(a, dtype=np.float32)
    x = f(inputs["x"])
    # fold bn scales into weights (bn(conv(x,W),s,b) = conv(x, s*W) + b)
    w1 = f(inputs["w_conv1"]) * f(inputs["bn1_s"])[:, None]
    wq = f(inputs["wq"]) * f(inputs["bnq_s"])[:, None]
    wk = f(inputs["wk"]) * f(inputs["bnk_s"])[:, None]
    # fold bnatt scale through the (linear) attention-value path into v
    sv = f(inputs["bnatt_s"]) * f(inputs["bnv_s"])
    wv = f(inputs["wv"]) * sv[:, None]
    bv = f(inputs["bnatt_s"]) * f(inputs["bnv_b"])
    w3 = f(inputs["w_conv3"]) * f(inputs["bn3_s"])[:, None]

    posf = (f(inputs["pos_h"]) + f(inputs["pos_w"])).reshape(WIDTH, NKK)

    def pt_major(w, ko, no):  # [no, ko] -> [P, ko/P, no] partition-major
        return np.ascontiguousarray(
            w.T.reshape(ko // P, P, no).transpose(1, 0, 2))

    sel = np.zeros((PT, P, HEADS), np.float32)
    for pt in range(PT):
        for c in range(P):
            sel[pt, c, pt * (P // D) + c // D] = 1.0
    sab = np.zeros((P, HEADS), np.float32)
    for r in range(P):
        sab[r, r % HEADS] = 1.0
    # p2[p, pt, di, 32*dj+g] = pos[pt*128+p, 3di+dj] if head(pt*128+p)==g
    p2 = np.zeros((PT, P, KS, RQ), np.float32)
    for pt in range(PT):
        for c in range(P):
            g = pt * (P // D) + c // D
            for kk in range(NKK):
                di, dj = kk // KS, kk % KS
                p2[pt, c, di, 32 * dj + g % HEADS] = posf[pt * P + c, kk]
    p2 = np.ascontiguousarray(p2.transpose(1, 0, 2, 3))

    # h2 relu/recip commute relies on zero bnatt bias (see vside tail)
    assert np.allclose(f(inputs["bnatt_b"]), 0.0), "bnatt_b must be zero"
    cstf = np.zeros((P, CF_N), np.float32)
    for name, arr, npt in (("b1", f(inputs["bn1_b"]), PT),
                           ("bq", f(inputs["bnq_b"]), PT),
                           ("bk", f(inputs["bnk_b"]), PT),
                           ("bv", bv, PT),
                           ("batt", f(inputs["bnatt_b"]), PT),
                           ("b3", f(inputs["bn3_b"]), OC)):
        cstf[:, _CF[name]:_CF[name] + npt] = arr.reshape(npt, P).T

    cstb = np.zeros((P, CB_N), np.float32)
    cstb[:, _CB["sel"]:_CB["sel"] + PT * HEADS] = (
        sel.transpose(1, 0, 2).reshape(P, PT * HEADS))
    cstb[:, _CB["sab"]:_CB["sab"] + HEADS] = sab
    cstb[:, _CB["ident"]:_CB["ident"] + P] = np.eye(P)

    com = {
        "w1T": pt_major(w1, CIN, WIDTH).astype(bf),
        "wqT": pt_major(wq, WIDTH, WIDTH).astype(bf),
        "wkT": pt_major(wk, WIDTH, WIDTH).astype(bf),
        "wvT": pt_major(wv, WIDTH, WIDTH).astype(bf),
        "w3T": pt_major(w3, WIDTH, OUT).astype(bf),
        "p2": p2.astype(bf),
        "cstf": cstf,
        "cstb": cstb.astype(bf),
    }
    xr = x.reshape(B, KC1, P, HW)
    in_maps = []
    for c in range(NC_):
        xs = np.ascontiguousarray(xr[c * BL:(c + 1) * BL])
        in_maps.append(dict(com, x16=xs.astype(bf)))
    return in_maps


def _finish(raw_outs, x):
    """Host-side tail: y = relu(z + x) with z the device output (conv3+b3)."""
    z = np.concatenate(
        [o.astype(np.float32).reshape(BL, OUT, H, W) for o in raw_outs], axis=0)
    return np.maximum(z + np.asarray(x, dtype=np.float32), 0.0)


def kernel(**inputs):
    global _PROG
    if _PROG is None:
        _PROG = build_program()
    in_maps = _host_prep(inputs)
    res = run_bass_kernel_spmd(_PROG, in_maps, core_ids=list(range(NC_)))
    return _finish([res.results[c]["out"] for c in range(NC_)], inputs["x"])
